# revision 1
# baseline (speedup 1.0000x reference)
"""Trainium2 Bass kernel for nn_DensePoseV1ConvXGNSparseHead.

8 layers of submanifold 3x3 conv (gather-GEMM over 9 taps) + GroupNorm(32)
+ ReLU on N=131072 sparse sites, 256->512 then 512->512 channels.

Strategy
--------
The 9-tap rulebook is a 3x3 stencil on a ~60%-occupied grid.  On the host we
reconstruct a planar embedding of the points from nbr_idx (min-label
propagation over the neighbor graph), pack the connected components into a
padded dense canvas, and run the conv as *dense* channel-major fp16 matmuls
with shifted access patterns: zero gather, zero transposes, contiguous DMA.
Inactive/pad cells are kept at exactly 0 by folding an activity mask into
the GroupNorm affine application, so submanifold semantics are preserved.

Sharding: canvas rows are split across the 8 cores with an 8-row halo on
each side - the full receptive field of 8 stacked 3x3 convs - so every core
computes its slice for all 8 layers with ZERO inter-core communication.

v2 (this file): conv path in fp16 (halves LDWEIGHTS passes + DMA bytes and
lowers PE power -> less gpio/P0 throttling), per-layer weight double-buffer
with cross-layer prefetch, For_i unrolled 4 blocks per iteration (halves
all-engine loop barriers).  GroupNorm statistics math stays fp32.
"""

import numpy as np

import concourse.bass as bass
import concourse.tile as tile
from concourse import bacc, mybir
from concourse.bass_utils import run_bass_kernel_spmd

DT = mybir.dt
F16 = DT.float16

N_TAPS = 9
OFFS = [(dy, dx) for dy in (-1, 0, 1) for dx in (-1, 0, 1)]
OFFS_ARR = np.array(OFFS, np.int64)
HALO_ROWS = 8
N_CORES = 8
BLOCK = 1536
SUB = 512  # psum subtile (fp32 bank)
NSUB = BLOCK // SUB
HA = 1024  # first conv half (psum tag ca, 2 banks); second half is SUB
HID = 512
GSIZE = 16
EPS = 1e-5
CPAD = 128  # zero columns left/right of the compute region (conv reads +-67)
WIN = BLOCK + 2 * 67  # x window per block


# ----------------------------------------------------------------- host side

def _embed_points(nbr):
    n = nbr.shape[0]
    assert nbr.shape[1] == N_TAPS
    assert (nbr[:, 4] == np.arange(n)).all(), "tap 4 must be self"
    comp = np.arange(n, dtype=np.int64)
    py = np.zeros(n, np.int64)
    px = np.zeros(n, np.int64)
    edges = []
    for k in range(N_TAPS):
        if k == 4:
            continue
        t = nbr[:, k]
        src = np.flatnonzero(t >= 0)
        edges.append((src, t[src].astype(np.int64), int(OFFS_ARR[k, 0]),
                      int(OFFS_ARR[k, 1])))
    for _ in range(100_000):
        changed = False
        for src, dst, dy, dx in edges:
            bad = comp[src] < comp[dst]
            if bad.any():
                s, d = src[bad], dst[bad]
                order = np.argsort(comp[s], kind="stable")
                s, d = s[order], d[order]
                uniq, first = np.unique(d, return_index=True)
                s, d = s[first], uniq
                comp[d] = comp[s]
                py[d] = py[s] + dy
                px[d] = px[s] + dx
                changed = True
        if not changed:
            break
    else:
        raise RuntimeError("label propagation did not converge")
    for k in range(N_TAPS):
        t = nbr[:, k]
        src = np.flatnonzero(t >= 0)
        dst = t[src]
        ok = ((comp[src] == comp[dst])
              & (py[dst] == py[src] + OFFS_ARR[k, 0])
              & (px[dst] == px[src] + OFFS_ARR[k, 1]))
        if not ok.all():
            raise RuntimeError(f"rulebook inconsistent at tap {k}")
    return comp, py, px


def _build_canvas_map(nbr):
    n = nbr.shape[0]
    comp, py, px = _embed_points(nbr)
    uniq, inv = np.unique(comp, return_inverse=True)
    ncmp = uniq.size
    big = 1 << 60
    miny = np.full(ncmp, big); minx = np.full(ncmp, big)
    maxy = np.full(ncmp, -big); maxx = np.full(ncmp, -big)
    np.minimum.at(miny, inv, py); np.minimum.at(minx, inv, px)
    np.maximum.at(maxy, inv, py); np.maximum.at(maxx, inv, px)
    h = maxy - miny + 1
    w = maxx - minx + 1
    stride = int(w.max()) + 2
    shelf_w = stride - 2

    # Pack components: big ones stacked vertically (full rows); small ones
    # shelf-packed side by side to avoid burning a full canvas row each.
    npts = np.bincount(inv)
    isbig = npts > 1000
    row_off = np.zeros(ncmp, np.int64)
    col_off = np.ones(ncmp, np.int64)
    acc = 0
    for c in np.flatnonzero(isbig):
        row_off[c] = acc
        acc += int(h[c]) + 1
    order = sorted(np.flatnonzero(~isbig), key=lambda c: -int(h[c]))
    shelf_row, shelf_h, xcur = acc, 0, 0
    for c in order:
        if xcur + int(w[c]) > shelf_w:
            shelf_row += shelf_h + 1
            shelf_h, xcur = 0, 0
        if shelf_h == 0:
            shelf_h = int(h[c])
        row_off[c] = shelf_row
        col_off[c] = 1 + xcur
        xcur += int(w[c]) + 1
    if xcur > 0:
        shelf_row += shelf_h + 1
    total_rows = int(shelf_row)
    r8 = -(-total_rows // N_CORES)
    rg = N_CORES * r8 + 2 * HALO_ROWS
    grow = HALO_ROWS + row_off[inv] + (py - miny[inv])
    gcol = col_off[inv] + (px - minx[inv])
    pos = grow * stride + gcol
    occupied = np.zeros(rg * stride, bool)
    if pos.max() >= occupied.size or np.unique(pos).size != n:
        raise RuntimeError("canvas build failed")
    for k in range(N_TAPS):
        if k == 4:
            continue
        occupied[:] = False
        occupied[pos] = True
        dpos = int(OFFS_ARR[k, 0]) * stride + int(OFFS_ARR[k, 1])
        if occupied[pos[nbr[:, k] < 0] + dpos].any():
            raise RuntimeError(f"tap {k}: active cell where rulebook says -1")
    m_raw = (r8 + 2 * HALO_ROWS) * stride
    m_pad = -(-m_raw // BLOCK) * BLOCK
    return pos, dict(stride=stride, r8=r8, rg=rg, m_raw=m_raw, m_pad=m_pad,
                     n_blocks=m_pad // BLOCK)


# --------------------------------------------------------------- bass program

def _build_program(m_pad, n_blocks, layers, stride, m_raw):
    # extra tail slack so the cross-block prefetch may harmlessly overread
    padw = CPAD + m_pad + CPAD + BLOCK + 128
    z0_start = CPAD + m_pad
    nc = bacc.Bacc("TRN2", target_bir_lowering=False, debug=False)

    x0_d = nc.dram_tensor("x0", (2, 128, padw), F16, kind="ExternalInput")
    w0_d = nc.dram_tensor("w0p", (128, N_TAPS * 2 * HID), F16,
                          kind="ExternalInput")
    wr_d = nc.dram_tensor("wrp", (max(layers - 1, 1), 128, N_TAPS * 4 * HID),
                          F16, kind="ExternalInput")
    acg_d = nc.dram_tensor("acg", (layers, 32, 2048), F16, kind="ExternalInput")
    gm_d = nc.dram_tensor("gm16", (layers, 32, 2048), F16, kind="ExternalInput")
    bc_d = nc.dram_tensor("bc32", (layers, 32, 2048), F16, kind="ExternalInput")
    smask_d = nc.dram_tensor("smask", (128, 128), F16, kind="ExternalInput")
    msk32_d = nc.dram_tensor("msk32", (32, m_pad // 3 + SUB), F16,
                             kind="ExternalInput")
    out_d = nc.dram_tensor("out", (4, 128, m_pad), DT.float32,
                           kind="ExternalOutput")
    xa_d = nc.dram_tensor("xa", (4, 128, padw), F16, kind="Internal")
    xb_d = nc.dram_tensor("xb", (4, 128, padw), F16, kind="Internal")

    deltas = [dy * stride + dx for dy, dx in OFFS]
    WCOLS = N_TAPS * 4 * HID

    with tile.TileContext(nc) as tc:
        with (
            tc.tile_pool(name="consts", bufs=1) as constp,
            tc.tile_pool(name="wp", bufs=2) as wpool,
            tc.tile_pool(name="lyc", bufs=2) as lycp,
            tc.tile_pool(name="xwp", bufs=1) as xwpool,
            tc.tile_pool(name="yb", bufs=3) as ypool,
            tc.tile_pool(name="yf", bufs=2) as yfpool,
            tc.tile_pool(name="ysq", bufs=2) as ysqpool,
            tc.tile_pool(name="tt", bufs=1) as ttpool,
            tc.tile_pool(name="tt2", bufs=2) as tt2pool,
            tc.tile_pool(name="psC", bufs=1, space=bass.MemorySpace.PSUM) as psCp,
            tc.tile_pool(name="psS", bufs=1, space=bass.MemorySpace.PSUM) as psSp,
            tc.tile_pool(name="psA", bufs=1, space=bass.MemorySpace.PSUM) as psAp,
            tc.tile_pool(name="psB", bufs=2, space=bass.MemorySpace.PSUM) as psBp,
        ):
            smask = constp.tile([128, 128], F16)
            nc.sync.dma_start(smask[:], smask_d.ap())
            xw0 = constp.tile([128, 4 * WIN], F16, tag="xw0")
            xw1 = constp.tile([128, 4 * WIN], F16, tag="xw1")
            msk0 = constp.tile([32, SUB], F16, tag="msk0")
            msk1 = constp.tile([32, SUB], F16, tag="msk1")
            xwt = [xw0, xw1]
            mskt = [msk0, msk1]

            # zero the conv pads of the internal ping-pong buffers once
            zpad = constp.tile([128, CPAD], F16)
            nc.gpsimd.memset(zpad[:], 0.0)
            for buf in (xa_d, xb_d):
                for ci in range(4):
                    nc.sync.dma_start(buf.ap()[ci, :, 0:CPAD], zpad[:])
                    for z0 in range(z0_start, padw, CPAD):
                        zw = min(CPAD, padw - z0)
                        nc.sync.dma_start(buf.ap()[ci, :, z0:z0 + zw],
                                          zpad[:, 0:zw])

            def load_weights(li):
                w_sb = wpool.tile([128, WCOLS], F16, tag="w")
                nci = 2 if li == 0 else 4
                wq = N_TAPS * nci * 128  # cols per co chunk
                wsrc = (w0_d.ap() if li == 0
                        else wr_d.ap()[li - 1, :, 0:WCOLS])
                for co in range(4):
                    nc.sync.dma_start(w_sb[:, co * wq:(co + 1) * wq],
                                      wsrc[:, co * wq:(co + 1) * wq])
                return w_sb

            def load_xw(p, src_aps, nci, bexpr):
                for ci in range(nci):
                    nc.sync.dma_start(
                        xwt[p][:, ci * WIN:(ci + 1) * WIN],
                        src_aps[ci][:, bass.ds(bexpr * BLOCK + (CPAD - 67),
                                               WIN)])
                nc.sync.dma_start(
                    mskt[p][:], msk32_d.ap()[:, bass.ds(bexpr * SUB, SUB)])

            def run_layer(li, nci, src_aps, dst_aps, final, w_sb, prefetch):
                def conv_half(co, j0, j1, ps):
                    nmm = nci * N_TAPS
                    mi = 0
                    for ci in range(nci):
                        for k in range(N_TAPS):
                            woff = (co * nci * N_TAPS + k * nci + ci) * 128
                            lhsT = w_sb[:, woff:woff + 128]
                            base = ci * WIN + 67 + deltas[k]
                            for j in range(j0, j1):
                                nc.tensor.matmul(
                                    ps[:, (j - j0) * SUB:(j - j0 + 1) * SUB],
                                    lhsT,
                                    xwt[cur][:, base + j * SUB:
                                              base + j * SUB + SUB],
                                    start=(mi == 0), stop=(mi == nmm - 1))
                            mi += 1

                def ep_stats(co, y, ysq, bexpr, jmax):
                    msk = mskt[cur]
                    psX = psSp.tile([32, SUB], DT.float32, tag="sx")
                    psXX = psSp.tile([32, SUB], DT.float32, tag="sxx")
                    acg = lycp.tile([32, SUB], F16, tag="acg")
                    nc.sync.dma_start(acg[:],
                                      acg_d.ap()[li, :, co * SUB:(co + 1) * SUB])
                    gm = lycp.tile([32, SUB], F16, tag="gm")
                    nc.sync.dma_start(gm[:],
                                      gm_d.ap()[li, :, co * SUB:(co + 1) * SUB])
                    bc = lycp.tile([32, SUB], F16, tag="bc")
                    nc.sync.dma_start(bc[:],
                                      bc_d.ap()[li, :, co * SUB:(co + 1) * SUB])
                    for j in range(jmax):
                        nc.tensor.matmul(psX[:],
                                         smask[:, j * 32:(j + 1) * 32],
                                         y[:, j * SUB:(j + 1) * SUB],
                                         start=(j == 0), stop=(j == jmax - 1))
                    for j in range(jmax):
                        nc.tensor.matmul(psXX[:],
                                         smask[:, j * 32:(j + 1) * 32],
                                         ysq[:, j * SUB:(j + 1) * SUB],
                                         start=(j == 0), stop=(j == jmax - 1))

                    sxs = ttpool.tile([32, SUB], DT.float32, tag="sxs")
                    nc.vector.tensor_copy(sxs[:], psX[:])
                    u = ttpool.tile([32, SUB], DT.float32, tag="u")
                    nc.vector.tensor_tensor(u[:], sxs[:], sxs[:],
                                            mybir.AluOpType.mult)
                    u2 = ttpool.tile([32, SUB], DT.float32, tag="u2")
                    nc.vector.tensor_scalar(u2[:], u[:], -1.0 / GSIZE, None,
                                            mybir.AluOpType.mult)
                    v = ttpool.tile([32, SUB], DT.float32, tag="v")
                    nc.vector.tensor_tensor(v[:], psXX[:], u2[:],
                                            mybir.AluOpType.add)
                    uu = ttpool.tile([32, SUB], DT.float32, tag="u")
                    nc.vector.tensor_scalar(uu[:], v[:], 1.0 / GSIZE, EPS,
                                            mybir.AluOpType.mult,
                                            mybir.AluOpType.add)
                    r = ttpool.tile([32, SUB], DT.float32, tag="r")
                    nc.vector.reciprocal_approx_fast(r[:], uu[:])
                    inv = ttpool.tile([32, SUB], DT.float32, tag="u2")
                    nc.scalar.activation(inv[:], r[:],
                                         mybir.ActivationFunctionType.Sqrt)
                    invm = tt2pool.tile([32, SUB], F16, tag="invm")
                    nc.vector.tensor_tensor(invm[:], inv[:], msk[:],
                                            mybir.AluOpType.mult)
                    w32 = tt2pool.tile([32, SUB], F16, tag="w32")
                    nc.vector.tensor_tensor(w32[:], sxs[:], invm[:],
                                            mybir.AluOpType.mult)
                    return invm, w32, msk, acg, gm, bc

                def ep_ab(co, y, invm, w32, msk, acg, gm, bc, bexpr, jmax):
                    yout = y
                    if final:
                        yout = yfpool.tile([128, BLOCK], DT.float32, tag="yf")
                    for j in range(jmax):
                        cj = j * 128
                        psA = psAp.tile([128, SUB], DT.float32, tag="A")
                        nc.tensor.matmul(psA[:], acg[:, cj:cj + 128],
                                         invm[:], start=True, stop=True)
                        psB = psBp.tile([128, SUB], DT.float32, tag="B")
                        nc.tensor.matmul(psB[:], bc[:, cj:cj + 128],
                                         msk[:], start=True, stop=False)
                        nc.tensor.matmul(psB[:], gm[:, cj:cj + 128],
                                         w32[:], start=False, stop=True)
                        t1 = tt2pool.tile([128, SUB], DT.float32, tag="t1")
                        nc.vector.tensor_tensor(
                            t1[:], psA[:], y[:, j * SUB:(j + 1) * SUB],
                            mybir.AluOpType.mult)
                        t2 = tt2pool.tile([128, SUB], DT.float32, tag="t2")
                        nc.vector.tensor_tensor(t2[:], psB[:], t1[:],
                                                mybir.AluOpType.add)
                        nc.scalar.activation(
                            yout[:, j * SUB:(j + 1) * SUB], t2[:],
                            mybir.ActivationFunctionType.Relu)

                    dst = dst_aps[co][:, bass.ds(bexpr * BLOCK + (0 if final
                                                                  else CPAD),
                                                 jmax * SUB)]
                    nc.sync.dma_start(dst, yout[:, 0:jmax * SUB])

                def run_block(bexpr, pre_bexpr, jmax=NSUB):
                    # prefetch next block's window while this one computes
                    load_xw(1 - cur, src_aps, nci, pre_bexpr)
                    pstat = []
                    pab = []
                    for co in range(4):
                        y = ypool.tile([128, BLOCK], F16, tag="y")
                        ysq = ysqpool.tile([128, BLOCK], F16, tag="ysq")
                        psa = psCp.tile([128, HA], DT.float32, tag="ca")
                        conv_half(co, 0, 2, psa)
                        nc.vector.tensor_copy(y[:, 0:HA], psa[:])
                        nc.vector.tensor_tensor(
                            ysq[:, 0:HA], y[:, 0:HA], y[:, 0:HA],
                            mybir.AluOpType.mult)
                        if jmax == NSUB:
                            psb = psCp.tile([128, SUB], DT.float32, tag="cb")
                            conv_half(co, 2, NSUB, psb)
                            nc.vector.tensor_copy(y[:, HA:BLOCK], psb[:])
                            nc.vector.tensor_tensor(
                                ysq[:, HA:BLOCK], y[:, HA:BLOCK],
                                y[:, HA:BLOCK], mybir.AluOpType.mult)
                        pstat.append((co, y, ysq))
                        if len(pstat) > 1:
                            c_, y_, ysq_ = pstat.pop(0)
                            st = ep_stats(c_, y_, ysq_, bexpr, jmax)
                            pab.append((c_, y_) + st)
                        if len(pab) > 1:
                            ep_ab(*pab.pop(0), bexpr, jmax)
                    while pstat:
                        c_, y_, ysq_ = pstat.pop(0)
                        st = ep_stats(c_, y_, ysq_, bexpr, jmax)
                        pab.append((c_, y_) + st)
                        while len(pab) > 1:
                            ep_ab(*pab.pop(0), bexpr, jmax)
                    while pab:
                        ep_ab(*pab.pop(0), bexpr, jmax)

                # prologue: window for block 0, then prefetch next layer's
                # weights into the other wpool buffer (overlaps this layer)
                cur = 0
                load_xw(0, src_aps, nci, 0)
                w_next = prefetch()
                nstep = 4
                nbe = n_blocks - (n_blocks % nstep)
                if nbe:
                    with tc.For_i(0, nbe, nstep,
                                  hint_engines=(mybir.EngineType.PE,)) as i:
                        for u in range(nstep):
                            run_block(i + u, i + u + 1)
                            cur = 1 - cur
                for t in range(nbe, n_blocks):
                    run_block(t, t + 1 if t + 1 < n_blocks else t)
                    cur = 1 - cur
                tc.strict_bb_all_engine_barrier()
                return w_next

            bufs = {"x0": x0_d, "xa": xa_d, "xb": xb_d}
            seq = ["x0"] + ["xa", "xb"] * 4
            w_sb = load_weights(0)
            for li in range(layers):
                src, dst = seq[li], seq[li + 1]
                nci = 2 if li == 0 else 4
                src_aps = [bufs[src].ap()[ci] for ci in range(nci)]
                final = li == layers - 1
                dst_aps = ([out_d.ap()[co] for co in range(4)] if final
                           else [bufs[dst].ap()[co] for co in range(4)])
                prefetch = ((lambda li=li: load_weights(li + 1))
                            if li + 1 < layers else (lambda: None))
                w_sb = run_layer(li, nci, src_aps, dst_aps, final, w_sb,
                                 prefetch)

    nc.compile()
    return nc


# ------------------------------------------------------------- host packing

def _pack_host(inputs, pos, meta, layers=8):
    feats = np.ascontiguousarray(np.asarray(inputs["features"], np.float32))
    w0 = np.asarray(inputs["w0"], np.float32)
    w_rest = np.asarray(inputs["w_rest"], np.float32)
    gamma = np.asarray(inputs["gamma"], np.float32)
    beta = np.asarray(inputs["beta"], np.float32)
    n, cin = feats.shape
    stride, r8, m_pad = meta["stride"], meta["r8"], meta["m_pad"]
    rgst = meta["rg"] * stride
    padw = CPAD + m_pad + CPAD + BLOCK + 128

    x_g = np.zeros((cin, rgst), np.float16)
    x_g[:, pos] = feats.T.astype(np.float16)
    mask_g = np.zeros(rgst, np.float16)
    mask_g[pos] = 1.0

    # weights: wpk[p, co, k, ci, :] = w[k, ci*128+p, co*128:(co+1)*128]
    w0p = np.ascontiguousarray(
        w0.reshape(N_TAPS, 2, 128, 4, 128).transpose(2, 3, 0, 1, 4)
    ).reshape(128, N_TAPS * 2 * HID).astype(np.float16)
    nl = max(layers - 1, 1)
    wrp = np.ascontiguousarray(
        w_rest[:layers - 1].reshape(layers - 1, N_TAPS, 4, 128, 4, 128)
        .transpose(0, 3, 4, 1, 2, 5)
    ).reshape(layers - 1, 128, N_TAPS * 4 * HID).astype(np.float16)
    if wrp.shape[0] < nl:
        wrp = np.zeros((nl, 128, N_TAPS * 4 * HID), np.float16)

    ch = np.arange(128)
    acg = np.zeros((layers, 32, 4, 4, 128), np.float32)
    gm16 = np.zeros((layers, 32, 4, 4, 128), np.float32)
    bc32 = np.zeros((layers, 32, 4, 4, 128), np.float32)
    for li in range(layers):
        for co in range(4):
            g_ = gamma[li, co * 128:(co + 1) * 128]
            b_ = beta[li, co * 128:(co + 1) * 128]
            for j in range(4):
                rows = 8 * j + ch // GSIZE
                acg[li, rows, co, j, ch] = g_
                gm16[li, rows, co, j, ch] = -g_ / GSIZE
                bc32[li, 8 * j, co, j, :] = b_
    acg = acg.reshape(layers, 32, 2048).astype(np.float16)
    gm16 = gm16.reshape(layers, 32, 2048).astype(np.float16)
    bc32 = bc32.reshape(layers, 32, 2048).astype(np.float16)

    smask = np.zeros((128, 4, 32), np.float16)
    for j in range(4):
        smask[ch, j, 8 * j + ch // GSIZE] = 1.0
    smask = smask.reshape(128, 128)

    in_maps = []
    for s in range(N_CORES):
        c0 = s * r8 * stride
        x0 = np.zeros((2, 128, padw), np.float16)
        seg = x_g[:, c0:min(c0 + m_pad, rgst)]
        x0[:, :, CPAD:CPAD + seg.shape[1]] = seg.reshape(2, 128, -1)
        mc = np.zeros(m_pad, np.float16)
        mseg = mask_g[c0:min(c0 + m_pad, rgst)]
        mc[:mseg.shape[0]] = mseg
        # msk32[8j+g, b*512+c] = mask[b*2048 + j*512 + c]
        m4 = mc.reshape(-1, NSUB, SUB)  # [nb, j, c]
        msk32 = np.zeros((32, m_pad // NSUB + SUB), np.float16)
        for j in range(NSUB):
            for g in range(8):
                msk32[8 * j + g, :m_pad // NSUB] = m4[:, j, :].reshape(-1)
        in_maps.append({
            "x0": x0, "w0p": w0p, "wrp": wrp, "acg": acg, "gm16": gm16,
            "bc32": bc32, "smask": smask, "msk32": msk32,
        })
    return in_maps


TRACE = False
LAST_RESULT = {}


def kernel(**inputs) -> np.ndarray:
    nbr = np.asarray(inputs["nbr_idx"])
    n = nbr.shape[0]
    pos, meta = _build_canvas_map(nbr)
    in_maps = _pack_host(inputs, pos, meta)
    nc = _build_program(meta["m_pad"], meta["n_blocks"], 8, meta["stride"],
                        meta["m_raw"])
    res = run_bass_kernel_spmd(nc, in_maps, list(range(N_CORES)), trace=TRACE)
    LAST_RESULT["exec_time_ns"] = res.exec_time_ns
    LAST_RESULT["profile_json"] = res.profile_json

    stride, r8 = meta["stride"], meta["r8"]
    row = pos // stride
    own = np.clip((row - HALO_ROWS) // r8, 0, N_CORES - 1)
    result = np.zeros((n, HID), np.float32)
    for s in range(N_CORES):
        sel = own == s
        local = pos[sel] - s * r8 * stride
        o = res.results[s]["out"]  # [4, 128, m_pad]
        result[sel] = o[:, :, local].reshape(HID, -1).T
    return result


if __name__ == "__main__":
    import reference

    inputs = reference.setup_inputs()
    out = kernel(**{k: np.asarray(v) for k, v in inputs.items()})
    exp = np.asarray(reference.reference(**inputs))
    err = np.linalg.norm(out - exp) / np.linalg.norm(exp)
    print(f"l2 rel err: {err:.3e}")



# revision 7
# speedup vs baseline: 1.2297x; 1.2297x over previous
"""Trainium2 Bass kernel for nn_DensePoseV1ConvXGNSparseHead.

8 layers of submanifold 3x3 conv (gather-GEMM over 9 taps) + GroupNorm(32)
+ ReLU on N=131072 sparse sites, 256->512 then 512->512 channels.

Strategy
--------
Host reconstructs a planar embedding of the points from nbr_idx, packs the
connected components into a padded dense canvas (stride 66), and runs the
conv as dense channel-major fp16 matmuls.  Inactive/pad cells are kept at
exactly 0 by folding an activity mask into the GroupNorm affine, preserving
submanifold semantics.  Canvas rows are split across the 8 cores with an
8-row halo (receptive field of 8 stacked 3x3 convs) - zero inter-core
communication.

v3 (this file): the 3x3 conv runs as 1-D Winograd F(4,3) along canvas rows:
output rows in groups of 4, 6 position-planes V = B4^T x (DVE row combines),
per-position GEMMs over (3 dx taps x ci chunks) with host-transformed
weights W' = G4 w, output rows Y = A4^T U (DVE).  MACs per cell drop from
36 to 18 per 128-out-chunk, which matters because the baseline was at the
power-throttled PE streaming roofline.  GroupNorm statistics stay fp32.
"""

import numpy as np

import concourse.bass as bass
import concourse.tile as tile
from concourse import bacc, mybir
from concourse.bass_utils import run_bass_kernel_spmd

DT = mybir.dt
F16 = DT.float16

N_TAPS = 9
OFFS = [(dy, dx) for dy in (-1, 0, 1) for dx in (-1, 0, 1)]
OFFS_ARR = np.array(OFFS, np.int64)
HALO_ROWS = 8
N_CORES = 8
HID = 512
GSIZE = 16
EPS = 1e-5
CPAD = 128

SW = 66          # canvas row stride (w.max()+2, asserted host-side)
GQ = 7           # group-rows per block
QB = GQ * SW     # 462 quad-cells per block (psum width)
CB = 4 * QB      # 1848 output cells per block
WROWS = 30       # x-window rows per block (4*GQ + 2)
WPITCH = 2192    # x-window plane pitch (fits 4D rearrange span)
VW = QB + 2      # V-plane width incl 1 garbage-safe edge col each side
NPOS = 6

B4T = np.array([
    [4, 0, -5, 0, 1, 0], [0, -4, -4, 1, 1, 0], [0, 4, -4, -1, 1, 0],
    [0, -2, -1, 2, 1, 0], [0, 2, -1, -2, 1, 0], [0, 4, 0, -5, 0, 1]],
    np.float32)
G4 = np.array([
    [1 / 4, 0, 0], [-1 / 6, -1 / 6, -1 / 6], [-1 / 6, 1 / 6, -1 / 6],
    [1 / 24, 1 / 12, 1 / 6], [1 / 24, -1 / 12, 1 / 6], [0, 0, 1]],
    np.float32)
A4T = np.array([
    [1, 1, 1, 1, 1, 0], [0, 1, -1, 2, -2, 0],
    [0, 1, 1, 4, 4, 0], [0, 1, -1, 8, -8, 1]], np.float32)


# ----------------------------------------------------------------- host side

def _embed_points(nbr):
    n = nbr.shape[0]
    assert nbr.shape[1] == N_TAPS
    assert (nbr[:, 4] == np.arange(n)).all(), "tap 4 must be self"
    comp = np.arange(n, dtype=np.int64)
    py = np.zeros(n, np.int64)
    px = np.zeros(n, np.int64)
    edges = []
    for k in range(N_TAPS):
        if k == 4:
            continue
        t = nbr[:, k]
        src = np.flatnonzero(t >= 0)
        edges.append((src, t[src].astype(np.int64), int(OFFS_ARR[k, 0]),
                      int(OFFS_ARR[k, 1])))
    for _ in range(100_000):
        changed = False
        for src, dst, dy, dx in edges:
            bad = comp[src] < comp[dst]
            if bad.any():
                s, d = src[bad], dst[bad]
                order = np.argsort(comp[s], kind="stable")
                s, d = s[order], d[order]
                uniq, first = np.unique(d, return_index=True)
                s, d = s[first], uniq
                comp[d] = comp[s]
                py[d] = py[s] + dy
                px[d] = px[s] + dx
                changed = True
        if not changed:
            break
    else:
        raise RuntimeError("label propagation did not converge")
    for k in range(N_TAPS):
        t = nbr[:, k]
        src = np.flatnonzero(t >= 0)
        dst = t[src]
        ok = ((comp[src] == comp[dst])
              & (py[dst] == py[src] + OFFS_ARR[k, 0])
              & (px[dst] == px[src] + OFFS_ARR[k, 1]))
        if not ok.all():
            raise RuntimeError(f"rulebook inconsistent at tap {k}")
    return comp, py, px


def _build_canvas_map(nbr):
    n = nbr.shape[0]
    comp, py, px = _embed_points(nbr)
    uniq, inv = np.unique(comp, return_inverse=True)
    ncmp = uniq.size
    big = 1 << 60
    miny = np.full(ncmp, big); minx = np.full(ncmp, big)
    maxy = np.full(ncmp, -big); maxx = np.full(ncmp, -big)
    np.minimum.at(miny, inv, py); np.minimum.at(minx, inv, px)
    np.maximum.at(maxy, inv, py); np.maximum.at(maxx, inv, px)
    h = maxy - miny + 1
    w = maxx - minx + 1
    stride = int(w.max()) + 2
    assert stride == SW, f"stride {stride} != {SW}"
    shelf_w = stride - 2

    npts = np.bincount(inv)
    isbig = npts > 1000
    row_off = np.zeros(ncmp, np.int64)
    col_off = np.ones(ncmp, np.int64)
    acc = 0
    for c in np.flatnonzero(isbig):
        row_off[c] = acc
        acc += int(h[c]) + 1
    order = sorted(np.flatnonzero(~isbig), key=lambda c: -int(h[c]))
    shelf_row, shelf_h, xcur = acc, 0, 0
    for c in order:
        if xcur + int(w[c]) > shelf_w:
            shelf_row += shelf_h + 1
            shelf_h, xcur = 0, 0
        if shelf_h == 0:
            shelf_h = int(h[c])
        row_off[c] = shelf_row
        col_off[c] = 1 + xcur
        xcur += int(w[c]) + 1
    if xcur > 0:
        shelf_row += shelf_h + 1
    total_rows = int(shelf_row)
    # rows per core: multiple of 4 so core canvases align to quad groups
    r8 = -(-total_rows // N_CORES)
    r8 = -(-r8 // 4) * 4
    rg = N_CORES * r8 + 2 * HALO_ROWS
    grow = HALO_ROWS + row_off[inv] + (py - miny[inv])
    gcol = col_off[inv] + (px - minx[inv])
    pos = grow * stride + gcol
    occupied = np.zeros(rg * stride, bool)
    if pos.max() >= occupied.size or np.unique(pos).size != n:
        raise RuntimeError("canvas build failed")
    for k in range(N_TAPS):
        if k == 4:
            continue
        occupied[:] = False
        occupied[pos] = True
        dpos = int(OFFS_ARR[k, 0]) * stride + int(OFFS_ARR[k, 1])
        if occupied[pos[nbr[:, k] < 0] + dpos].any():
            raise RuntimeError(f"tap {k}: active cell where rulebook says -1")
    rows_core = r8 + 2 * HALO_ROWS          # 456
    assert rows_core % 4 == 0
    ng = rows_core // 4                      # quad groups per core
    nb = -(-ng // GQ)                        # winograd blocks per core
    m_raw = rows_core * stride               # valid cells per core
    m_out = nb * CB                          # cells written per core
    return pos, dict(stride=stride, r8=r8, rg=rg, m_raw=m_raw,
                     ng=ng, n_blocks=nb, m_out=m_out)


# --------------------------------------------------------------- bass program

def _build_program(meta, layers=8):
    nb = meta["n_blocks"]
    m_raw = meta["m_raw"]
    m_out = meta["m_out"]
    # x buffers: room for the last block's window read + zero slack
    win_end = CPAD + (nb - 1) * CB - SW + WROWS * SW
    padw = max(CPAD + m_out, win_end) + 2 * CPAD
    z0_start = CPAD + m_raw
    nc = bacc.Bacc("TRN2", target_bir_lowering=False, debug=False)

    WTC = NPOS * 3  # 18 weight planes per layer

    x0_d = nc.dram_tensor("x0", (2, 128, padw), F16, kind="ExternalInput")
    w0_d = nc.dram_tensor("w0p", (128, WTC * 2 * HID), F16,
                          kind="ExternalInput")
    wr_d = nc.dram_tensor("wrp", (max(layers - 1, 1), 128, WTC * 4 * HID),
                          F16, kind="ExternalInput")
    acg_d = nc.dram_tensor("acg", (layers, 32, 2048), F16, kind="ExternalInput")
    gm_d = nc.dram_tensor("gm16", (layers, 32, 2048), F16, kind="ExternalInput")
    bc_d = nc.dram_tensor("bc32", (layers, 32, 2048), F16, kind="ExternalInput")
    smask_d = nc.dram_tensor("smask", (128, 128), F16, kind="ExternalInput")
    msk32_d = nc.dram_tensor("msk32", (32, nb * QB), F16,
                             kind="ExternalInput")
    out_d = nc.dram_tensor("out", (4, 128, m_out), DT.float32,
                           kind="ExternalOutput")
    xa_d = nc.dram_tensor("xa", (4, 128, padw), F16, kind="Internal")
    xb_d = nc.dram_tensor("xb", (4, 128, padw), F16, kind="Internal")

    with tile.TileContext(nc) as tc:
        with (
            tc.tile_pool(name="consts", bufs=1) as constp,
            tc.tile_pool(name="wp", bufs=1) as wpool,
            tc.tile_pool(name="lyc", bufs=2) as lycp,
            tc.tile_pool(name="vp", bufs=1) as vpool,
            tc.tile_pool(name="vt", bufs=1) as vtp,
            tc.tile_pool(name="yb", bufs=3) as ypool,
            tc.tile_pool(name="ysq", bufs=2) as ysqpool,
            tc.tile_pool(name="yf", bufs=1) as yfpool,
            tc.tile_pool(name="ot", bufs=1) as otpool,
            tc.tile_pool(name="tt", bufs=1) as ttpool,
            tc.tile_pool(name="tt2", bufs=2) as tt2pool,
            tc.tile_pool(name="psU", bufs=1, space=bass.MemorySpace.PSUM) as psUp,
            tc.tile_pool(name="psT", bufs=2, space=bass.MemorySpace.PSUM) as psTp,
            tc.tile_pool(name="psAB", bufs=1, space=bass.MemorySpace.PSUM) as psABp,
        ):
            smask = constp.tile([128, 128], F16)
            nc.sync.dma_start(smask[:], smask_d.ap())
            xw0 = constp.tile([128, 4 * WPITCH], F16, tag="xw0")
            xw1 = constp.tile([128, 4 * WPITCH], F16, tag="xw1")
            msk0 = constp.tile([32, QB], F16, tag="msk0")
            msk1 = constp.tile([32, QB], F16, tag="msk1")
            xwt = [xw0, xw1]
            mskt = [msk0, msk1]

            # zero the pads of the internal ping-pong buffers once
            zpad = constp.tile([128, CPAD], F16)
            nc.gpsimd.memset(zpad[:], 0.0)
            for buf in (xa_d, xb_d):
                for ci in range(4):
                    nc.sync.dma_start(buf.ap()[ci, :, 0:CPAD], zpad[:])
                    for z0 in range(z0_start, padw, CPAD):
                        zw = min(CPAD, padw - z0)
                        nc.sync.dma_start(buf.ap()[ci, :, z0:z0 + zw],
                                          zpad[:, 0:zw])

            # V plane tiles: fixed (pos, ci) tags; edge cols written once
            # and never again (their values only reach masked outputs)
            vtiles = {}
            for p in range(NPOS):
                for ci in range(4):
                    vtiles[(p, ci)] = vpool.tile([128, VW], F16,
                                                 tag=f"v{p}_{ci}",
                                                 name=f"v{p}_{ci}")
            for v in vtiles.values():
                nc.gpsimd.memset(v[:, 0:1], 0.0)
                nc.gpsimd.memset(v[:, VW - 1:VW], 0.0)

            def load_weights(li):
                # 18 (pos,dx) planes; tag-level deps let the next layer's
                # DMAs start as this layer's last reader of a plane retires
                nci = 2 if li == 0 else 4
                wq = nci * 4 * 128
                tiles = []
                for pd in range(WTC):
                    wsb = wpool.tile([128, 4 * 4 * 128], F16, tag=f"w{pd}",
                                     name=f"w{pd}")
                    src = (w0_d.ap() if li == 0 else wr_d.ap()[li - 1])
                    nc.sync.dma_start(wsb[:, 0:wq],
                                      src[:, pd * wq:(pd + 1) * wq])
                    tiles.append(wsb)
                return tiles

            def load_xw(pp, src_aps, nci, bexpr):
                for ci in range(nci):
                    nc.sync.dma_start(
                        xwt[pp][:, ci * WPITCH:ci * WPITCH + WROWS * SW],
                        src_aps[ci][:, bass.ds(bexpr * CB + (CPAD - SW),
                                               WROWS * SW)])
                nc.sync.dma_start(
                    mskt[pp][:], msk32_d.ap()[:, bass.ds(bexpr * QB, QB)])

            TT = mybir.AluOpType

            def run_layer(li, nci, src_aps, dst_aps, final, w_tiles):

                def v_build(cur):
                    xw = xwt[cur]

                    def d(ci, i):
                        # [128, GQ, SW] AP: window rows 4g+i, g in [0,GQ)
                        base = ci * WPITCH + i * SW
                        seg = xw[:, base:base + GQ * 4 * SW]
                        return seg.rearrange("p (g q c) -> p g q c",
                                             g=GQ, q=4)[:, :, 0, :]

                    tt = nc.vector.tensor_tensor
                    ts = nc.vector.tensor_scalar

                    def fl(t):
                        return t[:].rearrange("p (g c) -> p g c", g=GQ)

                    for ci in range(nci):
                        d0, d1, d2, d3, d4, d5 = (d(ci, i) for i in range(6))
                        s1 = vtp.tile([128, QB], F16, tag="s1", name="vt_s1")
                        s2 = vtp.tile([128, QB], F16, tag="s2", name="vt_s2")
                        s3 = vtp.tile([128, QB], F16, tag="s3", name="vt_s3")

                        def vout(p):
                            return vtiles[(p, ci)][:, 1:1 + QB].rearrange(
                                "p (g c) -> p g c", g=GQ)

                        # V1 = (d3+d4) - 4(d1+d2)
                        tt(fl(s1), d1, d2, TT.add)
                        tt(fl(s2), d3, d4, TT.add)
                        ts(fl(s3), fl(s1), -4.0, None, TT.mult)
                        tt(vout(1), fl(s2), fl(s3), TT.add)
                        # V2 = 4(d1-d2) + (d4-d3)
                        tt(fl(s1), d1, d2, TT.subtract)
                        tt(fl(s2), d4, d3, TT.subtract)
                        ts(fl(s3), fl(s1), 4.0, None, TT.mult)
                        tt(vout(2), fl(s3), fl(s2), TT.add)
                        # V3 = 2(d3-d1) + (d4-d2); V4 = (d4-d2) - 2(d3-d1)
                        tt(fl(s1), d3, d1, TT.subtract)
                        tt(fl(s2), d4, d2, TT.subtract)
                        ts(fl(s3), fl(s1), 2.0, None, TT.mult)
                        tt(vout(3), fl(s3), fl(s2), TT.add)
                        tt(vout(4), fl(s2), fl(s3), TT.subtract)
                        # V0 = 4 d0 - 5 d2 + d4
                        ts(fl(s1), d2, -5.0, None, TT.mult)
                        tt(fl(s2), fl(s1), d4, TT.add)
                        ts(fl(s1), d0, 4.0, None, TT.mult)
                        tt(vout(0), fl(s1), fl(s2), TT.add)
                        # V5 = 4 d1 - 5 d3 + d5
                        ts(fl(s1), d3, -5.0, None, TT.mult)
                        tt(fl(s2), fl(s1), d5, TT.add)
                        ts(fl(s1), d1, 4.0, None, TT.mult)
                        tt(vout(5), fl(s1), fl(s2), TT.add)

                def chain(pos, co, psu):
                    mi = 0
                    nmm = 3 * nci
                    for dx in range(3):
                        wt = w_tiles[pos * 3 + dx]
                        for ci in range(nci):
                            lhsT = wt[:, (co * nci + ci) * 128:
                                      (co * nci + ci) * 128 + 128]
                            rhs = vtiles[(pos, ci)][:, dx:dx + QB]
                            nc.tensor.matmul(psu[:], lhsT, rhs,
                                             start=(mi == 0),
                                             stop=(mi == nmm - 1))
                            mi += 1

                def conv_transform(co, y):
                    # 6 position chains through 4 psum banks, interleaved
                    # with the A4T output transform so banks recycle
                    tt = nc.vector.tensor_tensor
                    ts = nc.vector.tensor_scalar
                    ua = psUp.tile([128, QB], DT.float32, tag="Ua",
                                   name="psUa")
                    chain(1, co, ua)
                    ub = psUp.tile([128, QB], DT.float32, tag="Ub",
                                   name="psUb")
                    chain(2, co, ub)
                    c1 = otpool.tile([128, QB], DT.float32, tag="oc",
                                     name="ot_c")
                    nc.vector.tensor_copy(c1[:], ua[:])
                    t_s = otpool.tile([128, QB], DT.float32, tag="os",
                                      name="ot_s")
                    tt(t_s[:], c1[:], ub[:], TT.add)
                    t_d = otpool.tile([128, QB], DT.float32, tag="od",
                                      name="ot_d")
                    tt(t_d[:], c1[:], ub[:], TT.subtract)
                    uc = psUp.tile([128, QB], DT.float32, tag="Uc",
                                   name="psUc")
                    chain(3, co, uc)
                    ud = psUp.tile([128, QB], DT.float32, tag="Ud",
                                   name="psUd")
                    chain(4, co, ud)
                    c2 = otpool.tile([128, QB], DT.float32, tag="oc",
                                     name="ot_c2")
                    nc.vector.tensor_copy(c2[:], uc[:])
                    t_t = otpool.tile([128, QB], DT.float32, tag="ost",
                                      name="ot_t")
                    tt(t_t[:], c2[:], ud[:], TT.add)
                    t_u = otpool.tile([128, QB], DT.float32, tag="ou",
                                      name="ot_u")
                    tt(t_u[:], c2[:], ud[:], TT.subtract)
                    u0 = psUp.tile([128, QB], DT.float32, tag="Ua",
                                   name="psU0")
                    chain(0, co, u0)
                    u5 = psUp.tile([128, QB], DT.float32, tag="Ub",
                                   name="psU5")
                    chain(5, co, u5)

                    y4 = y[:].rearrange("p (g r c) -> p g r c", g=GQ, r=4)

                    def flq(t):
                        return t[:].rearrange("p (g c) -> p g c", g=GQ)

                    t_a = otpool.tile([128, QB], DT.float32, tag="oa",
                                      name="ot_a")
                    # Y0 = U0 + s + t
                    tt(t_a[:], u0[:], t_s[:], TT.add)
                    tt(y4[:, :, 0, :], flq(t_a), flq(t_t), TT.add)
                    # Y1 = d + 2u
                    ts(t_a[:], t_u[:], 2.0, None, TT.mult)
                    tt(y4[:, :, 1, :], flq(t_d), flq(t_a), TT.add)
                    # Y2 = s + 4t
                    ts(t_a[:], t_t[:], 4.0, None, TT.mult)
                    tt(y4[:, :, 2, :], flq(t_s), flq(t_a), TT.add)
                    # Y3 = d + 8u + U5
                    ts(t_a[:], t_u[:], 8.0, None, TT.mult)
                    tt(t_a[:], t_d[:], t_a[:], TT.add)
                    tt(y4[:, :, 3, :], flq(t_a), flq(u5[:]), TT.add)

                def ep_stats(co, y, ysq, cur):
                    msk = mskt[cur]
                    pst = psTp.tile([64, QB], DT.float32, tag="st",
                                    name="pst")
                    psX = pst[0:32, :]
                    psXX = pst[32:64, :]
                    acg = lycp.tile([32, 512], F16, tag="acg")
                    nc.sync.dma_start(
                        acg[:], acg_d.ap()[li, :, co * 512:(co + 1) * 512])
                    gm = lycp.tile([32, 512], F16, tag="gm")
                    nc.sync.dma_start(
                        gm[:], gm_d.ap()[li, :, co * 512:(co + 1) * 512])
                    bc = lycp.tile([32, 512], F16, tag="bc")
                    nc.sync.dma_start(
                        bc[:], bc_d.ap()[li, :, co * 512:(co + 1) * 512])
                    for j in range(4):
                        nc.tensor.matmul(psX,
                                         smask[:, j * 32:(j + 1) * 32],
                                         y[:, j * QB:(j + 1) * QB],
                                         start=(j == 0), stop=(j == 3))
                    for j in range(4):
                        nc.tensor.matmul(psXX,
                                         smask[:, j * 32:(j + 1) * 32],
                                         ysq[:, j * QB:(j + 1) * QB],
                                         start=(j == 0), stop=(j == 3))

                    sxs = ttpool.tile([32, QB], DT.float32, tag="sxs")
                    nc.vector.tensor_copy(sxs[:], psX)
                    u = ttpool.tile([32, QB], DT.float32, tag="u")
                    nc.vector.tensor_tensor(u[:], sxs[:], sxs[:], TT.mult)
                    u2 = ttpool.tile([32, QB], DT.float32, tag="u2")
                    nc.vector.tensor_scalar(u2[:], u[:], -1.0 / GSIZE, None,
                                            TT.mult)
                    v = ttpool.tile([32, QB], DT.float32, tag="v")
                    nc.vector.tensor_tensor(v[:], psXX, u2[:], TT.add)
                    uu = ttpool.tile([32, QB], DT.float32, tag="u")
                    nc.vector.tensor_scalar(uu[:], v[:], 1.0 / GSIZE, EPS,
                                            TT.mult, TT.add)
                    r = ttpool.tile([32, QB], DT.float32, tag="r")
                    nc.vector.reciprocal_approx_fast(r[:], uu[:])
                    inv = ttpool.tile([32, QB], DT.float32, tag="u2")
                    nc.scalar.activation(inv[:], r[:],
                                         mybir.ActivationFunctionType.Sqrt)
                    invm = tt2pool.tile([32, QB], F16, tag="invm")
                    nc.vector.tensor_tensor(invm[:], inv[:], msk[:], TT.mult)
                    w32 = tt2pool.tile([32, QB], F16, tag="w32")
                    nc.vector.tensor_tensor(w32[:], sxs[:], invm[:], TT.mult)
                    return invm, w32, msk, acg, gm, bc

                def ep_ab(co, y, invm, w32, msk, acg, gm, bc, bexpr):
                    if final:
                        yout = yfpool.tile([128, CB], DT.float32, tag="yf")
                    else:
                        yout = y  # relu written in place after t1 reads y
                    for j in range(4):
                        cj = j * 128
                        psA = psABp.tile([128, QB], DT.float32, tag="A",
                                         name="psA")
                        nc.tensor.matmul(psA[:], acg[:, cj:cj + 128],
                                         invm[:], start=True, stop=True)
                        psB = psABp.tile([128, QB], DT.float32, tag="B",
                                         name="psB")
                        nc.tensor.matmul(psB[:], bc[:, cj:cj + 128],
                                         msk[:], start=True, stop=False)
                        nc.tensor.matmul(psB[:], gm[:, cj:cj + 128],
                                         w32[:], start=False, stop=True)
                        t1 = tt2pool.tile([128, QB], DT.float32, tag="t1")
                        nc.vector.tensor_tensor(
                            t1[:], psA[:], y[:, j * QB:(j + 1) * QB],
                            TT.mult)
                        t2 = tt2pool.tile([128, QB], DT.float32, tag="t2")
                        nc.vector.tensor_tensor(t2[:], psB[:], t1[:], TT.add)
                        nc.scalar.activation(
                            yout[:, j * QB:(j + 1) * QB], t2[:],
                            mybir.ActivationFunctionType.Relu)

                    dst = dst_aps[co][:, bass.ds(bexpr * CB + (0 if final
                                                              else CPAD),
                                                 CB)]
                    nc.sync.dma_start(dst, yout[:])

                def run_block(bexpr, pre_bexpr, cur):
                    load_xw(1 - cur, src_aps, nci, pre_bexpr)
                    v_build(cur)
                    pstat = []
                    pab = []
                    for co in range(4):
                        y = ypool.tile([128, CB], F16, tag="y")
                        conv_transform(co, y)
                        ysq = ysqpool.tile([128, CB], F16, tag="ysq")
                        nc.vector.tensor_tensor(ysq[:], y[:], y[:], TT.mult)
                        pstat.append((co, y, ysq))
                        if len(pstat) > 1:
                            c_, y_, ysq_ = pstat.pop(0)
                            st = ep_stats(c_, y_, ysq_, cur)
                            pab.append((c_, y_) + st)
                        if len(pab) > 1:
                            ep_ab(*pab.pop(0), bexpr)
                    while pstat:
                        c_, y_, ysq_ = pstat.pop(0)
                        st = ep_stats(c_, y_, ysq_, cur)
                        pab.append((c_, y_) + st)
                        while len(pab) > 1:
                            ep_ab(*pab.pop(0), bexpr)
                    while pab:
                        ep_ab(*pab.pop(0), bexpr)

                cur = 0
                load_xw(0, src_aps, nci, 0)
                nstep = 2
                nbe = (nb - 1) - ((nb - 1) % nstep)
                if nbe:
                    with tc.For_i(0, nbe, nstep,
                                  hint_engines=(mybir.EngineType.PE,)) as i:
                        for u in range(nstep):
                            run_block(i + u, i + u + 1, cur)
                            cur = 1 - cur
                for t in range(nbe, nb):
                    run_block(t, t + 1 if t + 1 < nb else t, cur)
                    cur = 1 - cur
                w_next = (load_weights(li + 1) if li + 1 < layers else None)
                tc.strict_bb_all_engine_barrier()
                return w_next

            bufs = {"x0": x0_d, "xa": xa_d, "xb": xb_d}
            seq = ["x0"] + ["xa", "xb"] * 4
            w_tiles = load_weights(0)
            for li in range(layers):
                src, dst = seq[li], seq[li + 1]
                nci = 2 if li == 0 else 4
                src_aps = [bufs[src].ap()[ci] for ci in range(nci)]
                final = li == layers - 1
                dst_aps = ([out_d.ap()[co] for co in range(4)] if final
                           else [bufs[dst].ap()[co] for co in range(4)])
                w_tiles = run_layer(li, nci, src_aps, dst_aps, final, w_tiles)

    nc.compile()
    return nc


# ------------------------------------------------------------- host packing

def _pack_host(inputs, pos, meta, layers=8):
    feats = np.ascontiguousarray(np.asarray(inputs["features"], np.float32))
    w0 = np.asarray(inputs["w0"], np.float32)
    w_rest = np.asarray(inputs["w_rest"], np.float32)
    gamma = np.asarray(inputs["gamma"], np.float32)
    beta = np.asarray(inputs["beta"], np.float32)
    n, cin = feats.shape
    stride, r8 = meta["stride"], meta["r8"]
    m_raw, m_out, nb = meta["m_raw"], meta["m_out"], meta["n_blocks"]
    rgst = meta["rg"] * stride
    win_end = CPAD + (nb - 1) * CB - SW + WROWS * SW
    padw = max(CPAD + m_out, win_end) + 2 * CPAD

    x_g = np.zeros((cin, rgst), np.float16)
    x_g[:, pos] = feats.T.astype(np.float16)
    mask_g = np.zeros(rgst, np.float16)
    mask_g[pos] = 1.0

    # winograd-transformed weights: per (pos,dx) plane, cols (co, ci, ch)
    def pack_w(w, nci):
        # w: [9, Cin, 512] -> out [128, 18 * nci*4*128]
        cin_ = nci * 128
        planes = []
        for p in range(NPOS):
            for dx in range(3):
                wp = np.zeros((cin_, HID), np.float32)
                for dy in range(3):
                    wp += G4[p, dy] * w[3 * dy + dx]
                # lhsT chunks [128, 128] per (co, ci): part dim = ci part
                arr = wp.reshape(nci, 128, 4, 128).transpose(1, 2, 0, 3)
                # arr[p_part, co, ci, ch]
                planes.append(arr.reshape(128, nci * 4 * 128))
        return np.concatenate(planes, axis=1).astype(np.float16)

    w0p = pack_w(w0, 2)
    nl = max(layers - 1, 1)
    wrp = np.zeros((nl, 128, NPOS * 3 * 4 * HID), np.float16)
    for li in range(layers - 1):
        wrp[li] = pack_w(w_rest[li], 4)

    ch = np.arange(128)
    acg = np.zeros((layers, 32, 4, 4, 128), np.float32)
    gm16 = np.zeros((layers, 32, 4, 4, 128), np.float32)
    bc32 = np.zeros((layers, 32, 4, 4, 128), np.float32)
    for li in range(layers):
        for co in range(4):
            g_ = gamma[li, co * 128:(co + 1) * 128]
            b_ = beta[li, co * 128:(co + 1) * 128]
            for j in range(4):
                rows = 8 * j + ch // GSIZE
                acg[li, rows, co, j, ch] = g_
                gm16[li, rows, co, j, ch] = -g_ / GSIZE
                bc32[li, 8 * j, co, j, :] = b_
    acg = acg.reshape(layers, 32, 2048).astype(np.float16)
    gm16 = gm16.reshape(layers, 32, 2048).astype(np.float16)
    bc32 = bc32.reshape(layers, 32, 2048).astype(np.float16)

    smask = np.zeros((128, 4, 32), np.float16)
    for j in range(4):
        smask[ch, j, 8 * j + ch // GSIZE] = 1.0
    smask = smask.reshape(128, 128)

    in_maps = []
    for s in range(N_CORES):
        c0 = s * r8 * stride
        x0 = np.zeros((2, 128, padw), np.float16)
        seg = x_g[:, c0:min(c0 + m_raw, rgst)]
        x0[:, :, CPAD:CPAD + seg.shape[1]] = seg.reshape(2, 128, -1)
        mc = np.zeros(nb * CB, np.float16)
        mseg = mask_g[c0:min(c0 + m_raw, rgst)]
        mc[:mseg.shape[0]] = mseg
        # msk32[8j+g, b*QB + c] = mask[b*CB + j*QB + c]
        m4 = mc.reshape(nb, 4, QB)
        msk32 = np.zeros((32, nb * QB), np.float16)
        for j in range(4):
            for g in range(8):
                msk32[8 * j + g] = m4[:, j, :].reshape(-1)
        in_maps.append({
            "x0": x0, "w0p": w0p, "wrp": wrp, "acg": acg, "gm16": gm16,
            "bc32": bc32, "smask": smask, "msk32": msk32,
        })
    return in_maps


TRACE = False
LAST_RESULT = {}


def kernel(**inputs) -> np.ndarray:
    nbr = np.asarray(inputs["nbr_idx"])
    n = nbr.shape[0]
    pos, meta = _build_canvas_map(nbr)
    in_maps = _pack_host(inputs, pos, meta)
    nc = _build_program(meta)
    res = run_bass_kernel_spmd(nc, in_maps, list(range(N_CORES)), trace=TRACE)
    LAST_RESULT["exec_time_ns"] = res.exec_time_ns
    LAST_RESULT["profile_json"] = res.profile_json

    stride, r8 = meta["stride"], meta["r8"]
    row = pos // stride
    own = np.clip((row - HALO_ROWS) // r8, 0, N_CORES - 1)
    result = np.zeros((n, HID), np.float32)
    for s in range(N_CORES):
        sel = own == s
        local = pos[sel] - s * r8 * stride
        o = res.results[s]["out"]  # [4, 128, m_out]
        result[sel] = o[:, :, local].reshape(HID, -1).T
    return result


if __name__ == "__main__":
    import reference

    inputs = reference.setup_inputs()
    out = kernel(**{k: np.asarray(v) for k, v in inputs.items()})
    exp = np.asarray(reference.reference(**inputs))
    err = np.linalg.norm(out - exp) / np.linalg.norm(exp)
    print(f"l2 rel err: {err:.3e}")


# revision 9
# speedup vs baseline: 1.3959x; 1.1351x over previous
"""Trainium2 Bass kernel for nn_DensePoseV1ConvXGNSparseHead.

8 layers of submanifold 3x3 conv (gather-GEMM over 9 taps) + GroupNorm(32)
+ ReLU on N=131072 sparse sites, 256->512 then 512->512 channels.

Strategy
--------
Host reconstructs a planar embedding of the points from nbr_idx, packs the
connected components into a padded dense canvas (stride 66), and runs the
conv as dense channel-major fp16 matmuls.  Inactive/pad cells are kept at
exactly 0 by folding an activity mask into the GroupNorm affine, preserving
submanifold semantics.  Canvas rows are split across the 8 cores with an
8-row halo (receptive field of 8 stacked 3x3 convs) - zero inter-core
communication.

v3 (this file): the 3x3 conv runs as 1-D Winograd F(4,3) along canvas rows:
output rows in groups of 4, 6 position-planes V = B4^T x (DVE row combines),
per-position GEMMs over (3 dx taps x ci chunks) with host-transformed
weights W' = G4 w, output rows Y = A4^T U (DVE).  MACs per cell drop from
36 to 18 per 128-out-chunk, which matters because the baseline was at the
power-throttled PE streaming roofline.  GroupNorm statistics stay fp32.
"""

import numpy as np

import concourse.bass as bass
import concourse.tile as tile
from concourse import bacc, mybir
from concourse.bass_utils import run_bass_kernel_spmd

DT = mybir.dt
F16 = DT.float16

N_TAPS = 9
OFFS = [(dy, dx) for dy in (-1, 0, 1) for dx in (-1, 0, 1)]
OFFS_ARR = np.array(OFFS, np.int64)
HALO_ROWS = 8
N_CORES = 8
HID = 512
GSIZE = 16
EPS = 1e-5
CPAD = 128

SW = 66          # canvas row stride (w.max()+2, asserted host-side)
GQ = 7           # group-rows per block
QB = GQ * SW     # 462 quad-cells per block (psum width)
CB = 4 * QB      # 1848 output cells per block
WROWS = 30       # x-window rows per block (4*GQ + 2)
WPITCH = 2192    # x-window plane pitch (fits 4D rearrange span)
VW = QB + 2      # V-plane width incl 1 garbage-safe edge col each side
NPOS = 6

B4T = np.array([
    [4, 0, -5, 0, 1, 0], [0, -4, -4, 1, 1, 0], [0, 4, -4, -1, 1, 0],
    [0, -2, -1, 2, 1, 0], [0, 2, -1, -2, 1, 0], [0, 4, 0, -5, 0, 1]],
    np.float32)
G4 = np.array([
    [1 / 4, 0, 0], [-1 / 6, -1 / 6, -1 / 6], [-1 / 6, 1 / 6, -1 / 6],
    [1 / 24, 1 / 12, 1 / 6], [1 / 24, -1 / 12, 1 / 6], [0, 0, 1]],
    np.float32)
A4T = np.array([
    [1, 1, 1, 1, 1, 0], [0, 1, -1, 2, -2, 0],
    [0, 1, 1, 4, 4, 0], [0, 1, -1, 8, -8, 1]], np.float32)


# ----------------------------------------------------------------- host side

def _embed_points(nbr):
    n = nbr.shape[0]
    assert nbr.shape[1] == N_TAPS
    assert (nbr[:, 4] == np.arange(n)).all(), "tap 4 must be self"
    comp = np.arange(n, dtype=np.int64)
    py = np.zeros(n, np.int64)
    px = np.zeros(n, np.int64)
    edges = []
    for k in range(N_TAPS):
        if k == 4:
            continue
        t = nbr[:, k]
        src = np.flatnonzero(t >= 0)
        edges.append((src, t[src].astype(np.int64), int(OFFS_ARR[k, 0]),
                      int(OFFS_ARR[k, 1])))
    for _ in range(100_000):
        changed = False
        for src, dst, dy, dx in edges:
            bad = comp[src] < comp[dst]
            if bad.any():
                s, d = src[bad], dst[bad]
                order = np.argsort(comp[s], kind="stable")
                s, d = s[order], d[order]
                uniq, first = np.unique(d, return_index=True)
                s, d = s[first], uniq
                comp[d] = comp[s]
                py[d] = py[s] + dy
                px[d] = px[s] + dx
                changed = True
        if not changed:
            break
    else:
        raise RuntimeError("label propagation did not converge")
    for k in range(N_TAPS):
        t = nbr[:, k]
        src = np.flatnonzero(t >= 0)
        dst = t[src]
        ok = ((comp[src] == comp[dst])
              & (py[dst] == py[src] + OFFS_ARR[k, 0])
              & (px[dst] == px[src] + OFFS_ARR[k, 1]))
        if not ok.all():
            raise RuntimeError(f"rulebook inconsistent at tap {k}")
    return comp, py, px


def _build_canvas_map(nbr):
    n = nbr.shape[0]
    comp, py, px = _embed_points(nbr)
    uniq, inv = np.unique(comp, return_inverse=True)
    ncmp = uniq.size
    big = 1 << 60
    miny = np.full(ncmp, big); minx = np.full(ncmp, big)
    maxy = np.full(ncmp, -big); maxx = np.full(ncmp, -big)
    np.minimum.at(miny, inv, py); np.minimum.at(minx, inv, px)
    np.maximum.at(maxy, inv, py); np.maximum.at(maxx, inv, px)
    h = maxy - miny + 1
    w = maxx - minx + 1
    stride = int(w.max()) + 2
    assert stride == SW, f"stride {stride} != {SW}"
    shelf_w = stride - 2

    npts = np.bincount(inv)
    isbig = npts > 1000
    row_off = np.zeros(ncmp, np.int64)
    col_off = np.ones(ncmp, np.int64)
    acc = 0
    for c in np.flatnonzero(isbig):
        row_off[c] = acc
        acc += int(h[c]) + 1
    order = sorted(np.flatnonzero(~isbig), key=lambda c: -int(h[c]))
    shelf_row, shelf_h, xcur = acc, 0, 0
    for c in order:
        if xcur + int(w[c]) > shelf_w:
            shelf_row += shelf_h + 1
            shelf_h, xcur = 0, 0
        if shelf_h == 0:
            shelf_h = int(h[c])
        row_off[c] = shelf_row
        col_off[c] = 1 + xcur
        xcur += int(w[c]) + 1
    if xcur > 0:
        shelf_row += shelf_h + 1
    total_rows = int(shelf_row)
    # rows per core: multiple of 4 so core canvases align to quad groups
    r8 = -(-total_rows // N_CORES)
    r8 = -(-r8 // 4) * 4
    rg = N_CORES * r8 + 2 * HALO_ROWS
    grow = HALO_ROWS + row_off[inv] + (py - miny[inv])
    gcol = col_off[inv] + (px - minx[inv])
    pos = grow * stride + gcol
    occupied = np.zeros(rg * stride, bool)
    if pos.max() >= occupied.size or np.unique(pos).size != n:
        raise RuntimeError("canvas build failed")
    for k in range(N_TAPS):
        if k == 4:
            continue
        occupied[:] = False
        occupied[pos] = True
        dpos = int(OFFS_ARR[k, 0]) * stride + int(OFFS_ARR[k, 1])
        if occupied[pos[nbr[:, k] < 0] + dpos].any():
            raise RuntimeError(f"tap {k}: active cell where rulebook says -1")
    rows_core = r8 + 2 * HALO_ROWS          # 456
    assert rows_core % 4 == 0
    ng = rows_core // 4                      # quad groups per core
    nb = -(-ng // GQ)                        # winograd blocks per core
    m_raw = rows_core * stride               # valid cells per core
    m_out = nb * CB                          # cells written per core
    return pos, dict(stride=stride, r8=r8, rg=rg, m_raw=m_raw,
                     ng=ng, n_blocks=nb, m_out=m_out)


# --------------------------------------------------------------- bass program

def _build_program(meta, layers=8):
    nb = meta["n_blocks"]
    m_raw = meta["m_raw"]
    m_out = meta["m_out"]
    # x buffers: room for the last block's window read + zero slack
    win_end = CPAD + (nb - 1) * CB - SW + WROWS * SW
    padw = max(CPAD + m_out, win_end) + 2 * CPAD
    z0_start = CPAD + m_raw
    nc = bacc.Bacc("TRN2", target_bir_lowering=False, debug=False)

    WTC = NPOS * 3  # 18 weight planes per layer

    x0_d = nc.dram_tensor("x0", (2, 128, padw), F16, kind="ExternalInput")
    w0_d = nc.dram_tensor("w0p", (128, WTC * 2 * HID), F16,
                          kind="ExternalInput")
    wr_d = nc.dram_tensor("wrp", (max(layers - 1, 1), 128, WTC * 4 * HID),
                          F16, kind="ExternalInput")
    acg_d = nc.dram_tensor("acg", (layers, 32, 2048), F16, kind="ExternalInput")
    gm_d = nc.dram_tensor("gm16", (layers, 32, 2048), F16, kind="ExternalInput")
    bc_d = nc.dram_tensor("bc32", (layers, 32, 2048), F16, kind="ExternalInput")
    smask_d = nc.dram_tensor("smask", (128, 128), F16, kind="ExternalInput")
    msk32_d = nc.dram_tensor("msk32", (32, nb * QB), F16,
                             kind="ExternalInput")
    out_d = nc.dram_tensor("out", (4, 128, m_out), DT.float32,
                           kind="ExternalOutput")
    xa_d = nc.dram_tensor("xa", (4, 128, padw), F16, kind="Internal")
    xb_d = nc.dram_tensor("xb", (4, 128, padw), F16, kind="Internal")

    with tile.TileContext(nc) as tc:
        with (
            tc.tile_pool(name="consts", bufs=1) as constp,
            tc.tile_pool(name="wp", bufs=1) as wpool,
            tc.tile_pool(name="lyc", bufs=2) as lycp,
            tc.tile_pool(name="vp", bufs=1) as vpool,
            tc.tile_pool(name="vt", bufs=1) as vtp,
            tc.tile_pool(name="yb", bufs=3) as ypool,
            tc.tile_pool(name="ysq", bufs=2) as ysqpool,
            tc.tile_pool(name="yf", bufs=1) as yfpool,
            tc.tile_pool(name="ot", bufs=1) as otpool,
            tc.tile_pool(name="tt", bufs=1) as ttpool,
            tc.tile_pool(name="tt2", bufs=2) as tt2pool,
            tc.tile_pool(name="psU", bufs=1, space=bass.MemorySpace.PSUM) as psUp,
            tc.tile_pool(name="psT", bufs=2, space=bass.MemorySpace.PSUM) as psTp,
            tc.tile_pool(name="psAB", bufs=1, space=bass.MemorySpace.PSUM) as psABp,
        ):
            smask = constp.tile([128, 128], F16)
            nc.sync.dma_start(smask[:], smask_d.ap())
            xw0 = constp.tile([128, 4, WPITCH], F16, tag="xw0")
            xw1 = constp.tile([128, 4, WPITCH], F16, tag="xw1")
            msk0 = constp.tile([32, QB], F16, tag="msk0")
            msk1 = constp.tile([32, QB], F16, tag="msk1")
            xwt = [xw0, xw1]
            mskt = [msk0, msk1]

            # zero the pads of the internal ping-pong buffers once
            zpad = constp.tile([128, CPAD], F16)
            nc.gpsimd.memset(zpad[:], 0.0)
            for buf in (xa_d, xb_d):
                for ci in range(4):
                    nc.sync.dma_start(buf.ap()[ci, :, 0:CPAD], zpad[:])
                    for z0 in range(z0_start, padw, CPAD):
                        zw = min(CPAD, padw - z0)
                        nc.sync.dma_start(buf.ap()[ci, :, z0:z0 + zw],
                                          zpad[:, 0:zw])

            # V plane tiles: fixed pos tags holding all 4 ci planes; edge
            # cols written once and never again (only reach masked outputs)
            vtiles = {}
            for p in range(NPOS):
                vtiles[p] = vpool.tile([128, 4, VW], F16, tag=f"v{p}",
                                       name=f"v{p}")
            for v in vtiles.values():
                for ci in range(4):
                    nc.gpsimd.memset(v[:, ci, 0:1], 0.0)
                    nc.gpsimd.memset(v[:, ci, VW - 1:VW], 0.0)

            def load_weights(li):
                # 18 (pos,dx) planes; tag-level deps let the next layer's
                # DMAs start as this layer's last reader of a plane retires
                nci = 2 if li == 0 else 4
                wq = nci * 4 * 128
                tiles = []
                for pd in range(WTC):
                    wsb = wpool.tile([128, 4 * 4 * 128], F16, tag=f"w{pd}",
                                     name=f"w{pd}")
                    src = (w0_d.ap() if li == 0 else wr_d.ap()[li - 1])
                    nc.sync.dma_start(wsb[:, 0:wq],
                                      src[:, pd * wq:(pd + 1) * wq])
                    tiles.append(wsb)
                return tiles

            def load_xw(pp, src_aps, nci, bexpr):
                for ci in range(nci):
                    nc.sync.dma_start(
                        xwt[pp][:, ci, 0:WROWS * SW],
                        src_aps[ci][:, bass.ds(bexpr * CB + (CPAD - SW),
                                               WROWS * SW)])
                nc.sync.dma_start(
                    mskt[pp][:], msk32_d.ap()[:, bass.ds(bexpr * QB, QB)])

            TT = mybir.AluOpType

            def run_layer(li, nci, src_aps, dst_aps, final, w_tiles):

                def v_build(cur):
                    xw = xwt[cur]

                    def d(i):
                        # 4D AP [128, nci, GQ, SW]: window rows 4g+i
                        seg = xw[:, 0:nci, i * SW:i * SW + GQ * 4 * SW]
                        return seg.rearrange("p ci (g q c) -> p ci g q c",
                                             g=GQ, q=4)[:, :, :, 0, :]

                    tt = nc.vector.tensor_tensor
                    CP = mybir.ActivationFunctionType.Copy

                    def sc(out, in_, k):
                        nc.scalar.activation(out, in_, CP, scale=float(k))

                    def fl(t):
                        return t[:].rearrange("p (ci g c) -> p ci g c",
                                              ci=4, g=GQ)[:, 0:nci]

                    def vout(p):
                        return vtiles[p][:, 0:nci, 1:1 + QB].rearrange(
                            "p ci (g c) -> p ci g c", g=GQ)

                    d0, d1, d2, d3, d4, d5 = (d(i) for i in range(6))
                    s1 = vtp.tile([128, 4 * QB], F16, tag="s1", name="vt_s1")
                    s2 = vtp.tile([128, 4 * QB], F16, tag="s2", name="vt_s2")
                    s3 = vtp.tile([128, 4 * QB], F16, tag="s3", name="vt_s3")
                    # V1 = (d3+d4) - 4(d1+d2)
                    tt(fl(s1), d1, d2, TT.add)
                    tt(fl(s2), d3, d4, TT.add)
                    sc(fl(s3), fl(s1), -4.0)
                    tt(vout(1), fl(s2), fl(s3), TT.add)
                    # V2 = 4(d1-d2) + (d4-d3)
                    tt(fl(s1), d1, d2, TT.subtract)
                    tt(fl(s2), d4, d3, TT.subtract)
                    sc(fl(s3), fl(s1), 4.0)
                    tt(vout(2), fl(s3), fl(s2), TT.add)
                    # V3 = 2(d3-d1) + (d4-d2); V4 = (d4-d2) - 2(d3-d1)
                    tt(fl(s1), d3, d1, TT.subtract)
                    tt(fl(s2), d4, d2, TT.subtract)
                    sc(fl(s3), fl(s1), 2.0)
                    tt(vout(3), fl(s3), fl(s2), TT.add)
                    tt(vout(4), fl(s2), fl(s3), TT.subtract)
                    # V0 = 4 d0 - 5 d2 + d4
                    sc(fl(s1), d2, -5.0)
                    tt(fl(s2), fl(s1), d4, TT.add)
                    sc(fl(s1), d0, 4.0)
                    tt(vout(0), fl(s1), fl(s2), TT.add)
                    # V5 = 4 d1 - 5 d3 + d5
                    sc(fl(s1), d3, -5.0)
                    tt(fl(s2), fl(s1), d5, TT.add)
                    sc(fl(s1), d1, 4.0)
                    tt(vout(5), fl(s1), fl(s2), TT.add)

                def chain(pos, co, psu):
                    mi = 0
                    nmm = 3 * nci
                    for dx in range(3):
                        wt = w_tiles[pos * 3 + dx]
                        for ci in range(nci):
                            lhsT = wt[:, (co * nci + ci) * 128:
                                      (co * nci + ci) * 128 + 128]
                            rhs = vtiles[pos][:, ci, dx:dx + QB]
                            nc.tensor.matmul(psu[:], lhsT, rhs,
                                             start=(mi == 0),
                                             stop=(mi == nmm - 1))
                            mi += 1

                def conv_transform(co, y):
                    # 6 position chains through 4 psum banks, interleaved
                    # with the A4T output transform so banks recycle
                    tt = nc.vector.tensor_tensor
                    ts = nc.vector.tensor_scalar
                    ua = psUp.tile([128, QB], DT.float32, tag="Ua",
                                   name="psUa")
                    chain(1, co, ua)
                    ub = psUp.tile([128, QB], DT.float32, tag="Ub",
                                   name="psUb")
                    chain(2, co, ub)
                    c1 = otpool.tile([128, QB], DT.float32, tag="oc",
                                     name="ot_c")
                    nc.vector.tensor_copy(c1[:], ua[:])
                    t_s = otpool.tile([128, QB], DT.float32, tag="os",
                                      name="ot_s")
                    tt(t_s[:], c1[:], ub[:], TT.add)
                    t_d = otpool.tile([128, QB], DT.float32, tag="od",
                                      name="ot_d")
                    tt(t_d[:], c1[:], ub[:], TT.subtract)
                    uc = psUp.tile([128, QB], DT.float32, tag="Uc",
                                   name="psUc")
                    chain(3, co, uc)
                    ud = psUp.tile([128, QB], DT.float32, tag="Ud",
                                   name="psUd")
                    chain(4, co, ud)
                    c2 = otpool.tile([128, QB], DT.float32, tag="oc",
                                     name="ot_c2")
                    nc.vector.tensor_copy(c2[:], uc[:])
                    t_t = otpool.tile([128, QB], DT.float32, tag="ost",
                                      name="ot_t")
                    tt(t_t[:], c2[:], ud[:], TT.add)
                    t_u = otpool.tile([128, QB], DT.float32, tag="ou",
                                      name="ot_u")
                    tt(t_u[:], c2[:], ud[:], TT.subtract)
                    u0 = psUp.tile([128, QB], DT.float32, tag="Ua",
                                   name="psU0")
                    chain(0, co, u0)
                    u5 = psUp.tile([128, QB], DT.float32, tag="Ub",
                                   name="psU5")
                    chain(5, co, u5)

                    y4 = y[:].rearrange("p (g r c) -> p g r c", g=GQ, r=4)

                    def flq(t):
                        return t[:].rearrange("p (g c) -> p g c", g=GQ)

                    t_a = otpool.tile([128, QB], DT.float32, tag="oa",
                                      name="ot_a")
                    # Y0 = U0 + s + t
                    tt(t_a[:], u0[:], t_s[:], TT.add)
                    tt(y4[:, :, 0, :], flq(t_a), flq(t_t), TT.add)
                    # Y1 = d + 2u
                    ts(t_a[:], t_u[:], 2.0, None, TT.mult)
                    tt(y4[:, :, 1, :], flq(t_d), flq(t_a), TT.add)
                    # Y2 = s + 4t
                    ts(t_a[:], t_t[:], 4.0, None, TT.mult)
                    tt(y4[:, :, 2, :], flq(t_s), flq(t_a), TT.add)
                    # Y3 = d + 8u + U5
                    ts(t_a[:], t_u[:], 8.0, None, TT.mult)
                    tt(t_a[:], t_d[:], t_a[:], TT.add)
                    tt(y4[:, :, 3, :], flq(t_a), flq(u5[:]), TT.add)

                def ep_stats(co, y, ysq, cur):
                    msk = mskt[cur]
                    pst = psTp.tile([64, QB], DT.float32, tag="st",
                                    name="pst")
                    psX = pst[0:32, :]
                    psXX = pst[32:64, :]
                    acg = lycp.tile([32, 512], F16, tag="acg")
                    nc.sync.dma_start(
                        acg[:], acg_d.ap()[li, :, co * 512:(co + 1) * 512])
                    gm = lycp.tile([32, 512], F16, tag="gm")
                    nc.sync.dma_start(
                        gm[:], gm_d.ap()[li, :, co * 512:(co + 1) * 512])
                    bc = lycp.tile([32, 512], F16, tag="bc")
                    nc.sync.dma_start(
                        bc[:], bc_d.ap()[li, :, co * 512:(co + 1) * 512])
                    for j in range(4):
                        nc.tensor.matmul(psX,
                                         smask[:, j * 32:(j + 1) * 32],
                                         y[:, j * QB:(j + 1) * QB],
                                         start=(j == 0), stop=(j == 3))
                    for j in range(4):
                        nc.tensor.matmul(psXX,
                                         smask[:, j * 32:(j + 1) * 32],
                                         ysq[:, j * QB:(j + 1) * QB],
                                         start=(j == 0), stop=(j == 3))

                    sxs = ttpool.tile([32, QB], DT.float32, tag="sxs")
                    nc.vector.tensor_copy(sxs[:], psX)
                    u = ttpool.tile([32, QB], DT.float32, tag="u")
                    nc.vector.tensor_tensor(u[:], sxs[:], sxs[:], TT.mult)
                    u2 = ttpool.tile([32, QB], DT.float32, tag="u2")
                    nc.vector.tensor_scalar(u2[:], u[:], -1.0 / GSIZE, None,
                                            TT.mult)
                    v = ttpool.tile([32, QB], DT.float32, tag="v")
                    nc.vector.tensor_tensor(v[:], psXX, u2[:], TT.add)
                    uu = ttpool.tile([32, QB], DT.float32, tag="u")
                    nc.vector.tensor_scalar(uu[:], v[:], 1.0 / GSIZE, EPS,
                                            TT.mult, TT.add)
                    r = ttpool.tile([32, QB], DT.float32, tag="r")
                    nc.vector.reciprocal_approx_fast(r[:], uu[:])
                    inv = ttpool.tile([32, QB], DT.float32, tag="u2")
                    nc.scalar.activation(inv[:], r[:],
                                         mybir.ActivationFunctionType.Sqrt)
                    invm = tt2pool.tile([32, QB], F16, tag="invm")
                    nc.vector.tensor_tensor(invm[:], inv[:], msk[:], TT.mult)
                    w32 = tt2pool.tile([32, QB], F16, tag="w32")
                    nc.vector.tensor_tensor(w32[:], sxs[:], invm[:], TT.mult)
                    return invm, w32, msk, acg, gm, bc

                def ep_ab(co, y, invm, w32, msk, acg, gm, bc, bexpr):
                    if final:
                        yout = yfpool.tile([128, CB], DT.float32, tag="yf")
                    else:
                        yout = y  # relu written in place after t1 reads y
                    for j in range(4):
                        cj = j * 128
                        psA = psABp.tile([128, QB], DT.float32, tag="A",
                                         name="psA")
                        nc.tensor.matmul(psA[:], acg[:, cj:cj + 128],
                                         invm[:], start=True, stop=True)
                        psB = psABp.tile([128, QB], DT.float32, tag="B",
                                         name="psB")
                        nc.tensor.matmul(psB[:], bc[:, cj:cj + 128],
                                         msk[:], start=True, stop=False)
                        nc.tensor.matmul(psB[:], gm[:, cj:cj + 128],
                                         w32[:], start=False, stop=True)
                        t1 = tt2pool.tile([128, QB], DT.float32, tag="t1")
                        nc.vector.tensor_tensor(
                            t1[:], psA[:], y[:, j * QB:(j + 1) * QB],
                            TT.mult)
                        t2 = tt2pool.tile([128, QB], DT.float32, tag="t2")
                        nc.vector.tensor_tensor(t2[:], psB[:], t1[:], TT.add)
                        nc.scalar.activation(
                            yout[:, j * QB:(j + 1) * QB], t2[:],
                            mybir.ActivationFunctionType.Relu)

                    dst = dst_aps[co][:, bass.ds(bexpr * CB + (0 if final
                                                              else CPAD),
                                                 CB)]
                    nc.sync.dma_start(dst, yout[:])

                def run_block(bexpr, pre_bexpr, cur):
                    load_xw(1 - cur, src_aps, nci, pre_bexpr)
                    v_build(cur)
                    pstat = []
                    pab = []
                    for co in range(4):
                        y = ypool.tile([128, CB], F16, tag="y")
                        conv_transform(co, y)
                        ysq = ysqpool.tile([128, CB], F16, tag="ysq")
                        nc.vector.tensor_tensor(ysq[:], y[:], y[:], TT.mult)
                        pstat.append((co, y, ysq))
                        if len(pstat) > 1:
                            c_, y_, ysq_ = pstat.pop(0)
                            st = ep_stats(c_, y_, ysq_, cur)
                            pab.append((c_, y_) + st)
                        if len(pab) > 1:
                            ep_ab(*pab.pop(0), bexpr)
                    while pstat:
                        c_, y_, ysq_ = pstat.pop(0)
                        st = ep_stats(c_, y_, ysq_, cur)
                        pab.append((c_, y_) + st)
                        while len(pab) > 1:
                            ep_ab(*pab.pop(0), bexpr)
                    while pab:
                        ep_ab(*pab.pop(0), bexpr)

                cur = 0
                load_xw(0, src_aps, nci, 0)
                nstep = 4
                nbe = (nb - 1) - ((nb - 1) % nstep)
                if nbe:
                    with tc.For_i(0, nbe, nstep,
                                  hint_engines=(mybir.EngineType.PE,)) as i:
                        for u in range(nstep):
                            run_block(i + u, i + u + 1, cur)
                            cur = 1 - cur
                for t in range(nbe, nb):
                    run_block(t, t + 1 if t + 1 < nb else t, cur)
                    cur = 1 - cur
                w_next = (load_weights(li + 1) if li + 1 < layers else None)
                tc.strict_bb_all_engine_barrier()
                return w_next

            bufs = {"x0": x0_d, "xa": xa_d, "xb": xb_d}
            seq = ["x0"] + ["xa", "xb"] * 4
            w_tiles = load_weights(0)
            for li in range(layers):
                src, dst = seq[li], seq[li + 1]
                nci = 2 if li == 0 else 4
                src_aps = [bufs[src].ap()[ci] for ci in range(nci)]
                final = li == layers - 1
                dst_aps = ([out_d.ap()[co] for co in range(4)] if final
                           else [bufs[dst].ap()[co] for co in range(4)])
                w_tiles = run_layer(li, nci, src_aps, dst_aps, final, w_tiles)

    nc.compile()
    return nc


# ------------------------------------------------------------- host packing

def _pack_host(inputs, pos, meta, layers=8):
    feats = np.ascontiguousarray(np.asarray(inputs["features"], np.float32))
    w0 = np.asarray(inputs["w0"], np.float32)
    w_rest = np.asarray(inputs["w_rest"], np.float32)
    gamma = np.asarray(inputs["gamma"], np.float32)
    beta = np.asarray(inputs["beta"], np.float32)
    n, cin = feats.shape
    stride, r8 = meta["stride"], meta["r8"]
    m_raw, m_out, nb = meta["m_raw"], meta["m_out"], meta["n_blocks"]
    rgst = meta["rg"] * stride
    win_end = CPAD + (nb - 1) * CB - SW + WROWS * SW
    padw = max(CPAD + m_out, win_end) + 2 * CPAD

    x_g = np.zeros((cin, rgst), np.float16)
    x_g[:, pos] = feats.T.astype(np.float16)
    mask_g = np.zeros(rgst, np.float16)
    mask_g[pos] = 1.0

    # winograd-transformed weights: per (pos,dx) plane, cols (co, ci, ch)
    def pack_w(w, nci):
        # w: [9, Cin, 512] -> out [128, 18 * nci*4*128]
        cin_ = nci * 128
        planes = []
        for p in range(NPOS):
            for dx in range(3):
                wp = np.zeros((cin_, HID), np.float32)
                for dy in range(3):
                    wp += G4[p, dy] * w[3 * dy + dx]
                # lhsT chunks [128, 128] per (co, ci): part dim = ci part
                arr = wp.reshape(nci, 128, 4, 128).transpose(1, 2, 0, 3)
                # arr[p_part, co, ci, ch]
                planes.append(arr.reshape(128, nci * 4 * 128))
        return np.concatenate(planes, axis=1).astype(np.float16)

    w0p = pack_w(w0, 2)
    nl = max(layers - 1, 1)
    wrp = np.zeros((nl, 128, NPOS * 3 * 4 * HID), np.float16)
    for li in range(layers - 1):
        wrp[li] = pack_w(w_rest[li], 4)

    ch = np.arange(128)
    acg = np.zeros((layers, 32, 4, 4, 128), np.float32)
    gm16 = np.zeros((layers, 32, 4, 4, 128), np.float32)
    bc32 = np.zeros((layers, 32, 4, 4, 128), np.float32)
    for li in range(layers):
        for co in range(4):
            g_ = gamma[li, co * 128:(co + 1) * 128]
            b_ = beta[li, co * 128:(co + 1) * 128]
            for j in range(4):
                rows = 8 * j + ch // GSIZE
                acg[li, rows, co, j, ch] = g_
                gm16[li, rows, co, j, ch] = -g_ / GSIZE
                bc32[li, 8 * j, co, j, :] = b_
    acg = acg.reshape(layers, 32, 2048).astype(np.float16)
    gm16 = gm16.reshape(layers, 32, 2048).astype(np.float16)
    bc32 = bc32.reshape(layers, 32, 2048).astype(np.float16)

    smask = np.zeros((128, 4, 32), np.float16)
    for j in range(4):
        smask[ch, j, 8 * j + ch // GSIZE] = 1.0
    smask = smask.reshape(128, 128)

    in_maps = []
    for s in range(N_CORES):
        c0 = s * r8 * stride
        x0 = np.zeros((2, 128, padw), np.float16)
        seg = x_g[:, c0:min(c0 + m_raw, rgst)]
        x0[:, :, CPAD:CPAD + seg.shape[1]] = seg.reshape(2, 128, -1)
        mc = np.zeros(nb * CB, np.float16)
        mseg = mask_g[c0:min(c0 + m_raw, rgst)]
        mc[:mseg.shape[0]] = mseg
        # msk32[8j+g, b*QB + c] = mask[b*CB + j*QB + c]
        m4 = mc.reshape(nb, 4, QB)
        msk32 = np.zeros((32, nb * QB), np.float16)
        for j in range(4):
            for g in range(8):
                msk32[8 * j + g] = m4[:, j, :].reshape(-1)
        in_maps.append({
            "x0": x0, "w0p": w0p, "wrp": wrp, "acg": acg, "gm16": gm16,
            "bc32": bc32, "smask": smask, "msk32": msk32,
        })
    return in_maps


TRACE = False
LAST_RESULT = {}


def kernel(**inputs) -> np.ndarray:
    nbr = np.asarray(inputs["nbr_idx"])
    n = nbr.shape[0]
    pos, meta = _build_canvas_map(nbr)
    in_maps = _pack_host(inputs, pos, meta)
    nc = _build_program(meta)
    res = run_bass_kernel_spmd(nc, in_maps, list(range(N_CORES)), trace=TRACE)
    LAST_RESULT["exec_time_ns"] = res.exec_time_ns
    LAST_RESULT["profile_json"] = res.profile_json

    stride, r8 = meta["stride"], meta["r8"]
    row = pos // stride
    own = np.clip((row - HALO_ROWS) // r8, 0, N_CORES - 1)
    result = np.zeros((n, HID), np.float32)
    for s in range(N_CORES):
        sel = own == s
        local = pos[sel] - s * r8 * stride
        o = res.results[s]["out"]  # [4, 128, m_out]
        result[sel] = o[:, :, local].reshape(HID, -1).T
    return result


if __name__ == "__main__":
    import reference

    inputs = reference.setup_inputs()
    out = kernel(**{k: np.asarray(v) for k, v in inputs.items()})
    exp = np.asarray(reference.reference(**inputs))
    err = np.linalg.norm(out - exp) / np.linalg.norm(exp)
    print(f"l2 rel err: {err:.3e}")
